# revision 1
# baseline (speedup 1.0000x reference)
"""Trainium2 Bass kernel for nn_Device_Policy (segment_reduce).

Strategy (matches the sharding hint): shard the node axis N across 8
NeuronCores.  Each core holds a [N/8, 64] state shard, a [N/8, 128]
mpnn_forward shard and a [64, N/8] slice of the assignment mask.  Per
core we compute:
  - partial column sums / sums-of-squares of `state`  (for the global
    per-feature normalization),
  - the partial masked segment sum dse.T = sum_n mpnn[n,:]^T mask[:,n]
    via PE matmuls (mask transposed on-chip through the PE).
The [128,64] dse.T partial plus the [64]+[64] state stats are packed
into one [128,66] buffer and AllReduce'd across the 8 cores; every core
then runs the tiny replicated MLP head and writes the [64] output.
"""

import sys

if "/opt/trn_rl_repo" not in sys.path:
    sys.path.insert(0, "/opt/trn_rl_repo")

import numpy as np

import concourse.bacc as bacc
import concourse.bass as bass
import concourse.mybir as mybir
import concourse.tile as tile
from concourse import masks
from concourse.bass_utils import run_bass_kernel_spmd

NCORES = 8
N = 262144
F = 64
D = 64
DF = 32
H1 = 128
H2 = 64
NSH = N // NCORES          # nodes per core = 32768
NB = 1024                  # nodes per mask tile
NT = NSH // NB             # 32 mask tiles per core
EPS = 1e-6
SLOPE = 0.1
MASK_VIA_CAST = False      # SWDGE cast DMA vs int-mult bit trick

f32 = mybir.dt.float32
i32 = mybir.dt.int32
ADD = mybir.AluOpType.add
MUL = mybir.AluOpType.mult
SUB = mybir.AluOpType.subtract
AXY = mybir.AxisListType.XY
AX = mybir.AxisListType.X
LRELU = mybir.ActivationFunctionType.Lrelu
IDENT = mybir.ActivationFunctionType.Identity
SQUARE = mybir.ActivationFunctionType.Square
SQRT = mybir.ActivationFunctionType.Sqrt


def build_program(stage="full"):
    nc = bacc.Bacc(
        "TRN2",
        target_bir_lowering=False,
        debug=False,
        enable_asserts=False,
        num_devices=NCORES,
    )

    x_state = nc.dram_tensor("x_state", [NSH, F], f32, kind="ExternalInput")
    x_mpnn = nc.dram_tensor("x_mpnn", [NSH, H1], f32, kind="ExternalInput")
    x_mask = nc.dram_tensor("x_mask", [D, NSH], i32, kind="ExternalInput")
    x_dfsT = nc.dram_tensor("x_dfsT", [64, D], f32, kind="ExternalInput")
    x_w1T = nc.dram_tensor("x_w1T", [64, H1], f32, kind="ExternalInput")
    x_b1 = nc.dram_tensor("x_b1", [H1, 1], f32, kind="ExternalInput")
    x_w2T = nc.dram_tensor("x_w2T", [F, H1], f32, kind="ExternalInput")
    x_b2 = nc.dram_tensor("x_b2", [H1, 1], f32, kind="ExternalInput")
    x_w3Tp = nc.dram_tensor("x_w3Tp", [H1, 4 * H2], f32, kind="ExternalInput")
    x_b3 = nc.dram_tensor("x_b3", [H2, 1], f32, kind="ExternalInput")
    x_w4T = nc.dram_tensor("x_w4T", [H2, 1], f32, kind="ExternalInput")
    x_b4 = nc.dram_tensor("x_b4", [D, 1], f32, kind="ExternalInput")
    x_spred = nc.dram_tensor("x_spred", [F, 1], f32, kind="ExternalInput")
    x_mpred = nc.dram_tensor("x_mpred", [H1, 1], f32, kind="ExternalInput")
    y_out = nc.dram_tensor("y_out", [D], f32, kind="ExternalOutput")
    y_dbg = None
    if stage != "full":
        y_dbg = nc.dram_tensor("y_dbg", [128, 66], f32, kind="ExternalOutput")

    with tile.TileContext(nc) as tc:
        emit(nc, tc, x_state, x_mpnn, x_mask, x_dfsT, x_w1T, x_b1, x_w2T,
             x_b2, x_w3Tp, x_b3, x_w4T, x_b4, x_spred, x_mpred, y_out,
             stage=stage, y_dbg=y_dbg)

    nc.compile()
    return nc


def emit(nc, tc, x_state, x_mpnn, x_mask, x_dfsT, x_w1T, x_b1, x_w2T, x_b2,
         x_w3Tp, x_b3, x_w4T, x_b4, x_spred, x_mpred, y_out,
         stage="full", y_dbg=None):
    ctx_pools = []

    def pool(name, bufs, space="SBUF"):
        p = tc.tile_pool(name=name, bufs=bufs, space=space)
        ctx_pools.append(p)
        return p.__enter__()

    cpool = pool("const", 1)
    maskf_pool = pool("maskf", 4)
    maski_pool = pool("maski", 4)
    maskT_pool = pool("maskT", 4)
    mpnn_pool = pool("mpnn", 6)
    state_pool = pool("state", 4)
    sq_pool = pool("sq", 2)
    tmp_pool = pool("tmp", 2)
    acc_pool = pool("acc", 1)
    ep_pool = pool("ep", 1)
    psumT_pool = pool("psumT", 3, space="PSUM")
    dse_pool = pool("dsepsum", 2, space="PSUM")
    eppsum_pool = pool("eppsum", 2, space="PSUM")
    dram_pool = pool("dram", 1, space="DRAM")

    # ---- constants ----
    ident = cpool.tile([128, 128], f32, name="ident")
    masks.make_identity(nc, ident[:, :])
    ones = cpool.tile([128, 1], f32, name="ones")
    nc.vector.memset(ones[:, :], 1.0)
    zeros = cpool.tile([128, D], f32, name="zeros")
    nc.vector.memset(zeros[:, :], 0.0)

    dfsT = cpool.tile([64, D], f32, name="dfsT")
    nc.sync.dma_start(dfsT[:, :], x_dfsT[:, :])
    w1T = cpool.tile([64, H1], f32, name="w1T")
    nc.sync.dma_start(w1T[:, :], x_w1T[:, :])
    b1 = cpool.tile([H1, 1], f32, name="b1")
    nc.sync.dma_start(b1[:, :], x_b1[:, :])
    w2T = cpool.tile([F, H1], f32, name="w2T")
    nc.sync.dma_start(w2T[:, :], x_w2T[:, :])
    b2 = cpool.tile([H1, 1], f32, name="b2")
    nc.sync.dma_start(b2[:, :], x_b2[:, :])
    w3Tp = cpool.tile([H1, 4 * H2], f32, name="w3Tp")
    nc.sync.dma_start(w3Tp[:, :], x_w3Tp[:, :])
    b3 = cpool.tile([H2, 1], f32, name="b3")
    nc.sync.dma_start(b3[:, :], x_b3[:, :])
    w4T = cpool.tile([H2, 1], f32, name="w4T")
    nc.sync.dma_start(w4T[:, :], x_w4T[:, :])
    b4 = cpool.tile([D, 1], f32, name="b4")
    nc.sync.dma_start(b4[:, :], x_b4[:, :])
    spred = cpool.tile([F, 1], f32, name="spred")
    nc.sync.dma_start(spred[:, :], x_spred[:, :])
    mpred = cpool.tile([H1, 1], f32, name="mpred")
    nc.sync.dma_start(mpred[:, :], x_mpred[:, :])

    acc_s = acc_pool.tile([128, F], f32, name="acc_s", tag="acc_s")
    acc_q = acc_pool.tile([128, F], f32, name="acc_q", tag="acc_q")
    acc_d = acc_pool.tile([H1, D], f32, name="acc_d", tag="acc_d")
    nc.vector.memset(acc_s[:, :], 0.0)
    nc.vector.memset(acc_q[:, :], 0.0)
    nc.vector.memset(acc_d[:, :], 0.0)

    # ---- main loop over node tiles of NB=1024 ----
    for t in range(NT):
        # mask half-tiles [64, 512], both based at partition 0 (PE matmul
        # operands at base_partition 64 wedge the device — keep everything
        # at base 0).  SWDGE dma casts int32 -> f32.
        mhalves = []
        for h in range(2):
            if MASK_VIA_CAST:
                mask_f = maskf_pool.tile([64, 512], f32, name="mask_f",
                                         tag=f"mask_f{h}")
                nc.gpsimd.dma_start(
                    mask_f[:, :],
                    x_mask[:, t * NB + h * 512:t * NB + (h + 1) * 512])
            else:
                mask_i = maski_pool.tile([64, 512], i32, name="mask_i",
                                         tag=f"mask_i{h}")
                nc.scalar.dma_start(
                    mask_i[:, :],
                    x_mask[:, t * NB + h * 512:t * NB + (h + 1) * 512])
                mask_f = maskf_pool.tile([64, 512], f32, name="mask_f",
                                         tag=f"mask_f{h}")
                # int 0/1 * 0x3F800000 bit-assembles f32 0.0/1.0 exactly
                nc.vector.tensor_scalar(mask_f[:, :].bitcast(i32),
                                        mask_i[:, :], 0x3F800000, None,
                                        op0=MUL)
            mhalves.append(mask_f)

        # transpose [64,128] blocks through the PE -> psumT [128, 8*64]
        psumT = psumT_pool.tile([128, 512], f32, name="psumT", tag="psumT")
        for b in range(8):
            h, j = divmod(b, 4)
            nc.tensor.transpose(
                psumT[:, b * 64:(b + 1) * 64],
                mhalves[h][:, j * 128:(j + 1) * 128],
                ident[0:64, 0:64],
            )
        # copy in halves so the first four dse matmuls can start while the
        # second half is still draining out of PSUM
        maskT = maskT_pool.tile([128, 512], f32, name="maskT", tag="maskT")
        nc.vector.tensor_copy(maskT[:, 0:256], psumT[:, 0:256])
        nc.vector.tensor_copy(maskT[:, 256:512], psumT[:, 256:512])

        # mpnn tiles (2 x 512 nodes) and the dse.T matmuls
        psum_dse = dse_pool.tile([H1, D], f32, name="psum_dse", tag="psum_dse")
        for half in range(2):
            mp = mpnn_pool.tile([128, 512], f32, name="mp", tag="mp")
            n0 = t * NB + half * 512
            nc.sync.dma_start(
                mp[:, :].rearrange("p (b h) -> p b h", b=4),
                x_mpnn[n0:n0 + 512, :].rearrange("(b p) h -> p b h", p=128),
            )
            for j in range(4):
                b = half * 4 + j
                nc.tensor.matmul(
                    psum_dse[:, :],
                    lhsT=mp[:, j * 128:(j + 1) * 128],
                    rhs=maskT[:, b * 64:(b + 1) * 64],
                    start=(b == 0),
                    stop=(b == 7),
                )
        nc.vector.tensor_add(acc_d[:, :], acc_d[:, :], psum_dse[:, :])

        # state tile: partitions p, free (g=4, r=2, f=64); row = g*256+p*2+r
        st = state_pool.tile([128, 512], f32, name="st", tag="st")
        nc.scalar.dma_start(
            st[:, :].rearrange("p (g r f) -> p g r f", g=4, r=2),
            x_state[t * NB:(t + 1) * NB, :].rearrange(
                "(g p r) f -> p g r f", p=128, r=2),
        )
        # dense add-tree over the (g, r) blocks — much faster on DVE than
        # a strided tensor_reduce (free layout is (g r f), f innermost)
        def addtree(src, acc, pfx):
            t1 = tmp_pool.tile([128, 256], f32, name=f"{pfx}1", tag=f"{pfx}1")
            nc.vector.tensor_add(t1[:, :], src[:, 0:256], src[:, 256:512])
            t2 = tmp_pool.tile([128, 128], f32, name=f"{pfx}2", tag=f"{pfx}2")
            nc.vector.tensor_add(t2[:, :], t1[:, 0:128], t1[:, 128:256])
            t3 = tmp_pool.tile([128, F], f32, name=f"{pfx}3", tag=f"{pfx}3")
            nc.vector.tensor_add(t3[:, :], t2[:, 0:64], t2[:, 64:128])
            nc.vector.tensor_add(acc[:, :], acc[:, :], t3[:, :])

        addtree(st[:, :], acc_s, "ts")
        sq = sq_pool.tile([128, 512], f32, name="sq", tag="sq")
        nc.scalar.activation(sq[:, :], st[:, :], SQUARE)
        addtree(sq[:, :], acc_q, "tq")

    # ---- reduce state stats across partitions (transposed via PE) ----
    psum_sv = eppsum_pool.tile([F, 1], f32, name="psum_sv", tag="ep")
    nc.tensor.matmul(psum_sv[:, :], lhsT=acc_s[:, :], rhs=ones[:, :],
                     start=True, stop=True)
    psum_qv = eppsum_pool.tile([F, 1], f32, name="psum_qv", tag="ep")
    nc.tensor.matmul(psum_qv[:, :], lhsT=acc_q[:, :], rhs=ones[:, :],
                     start=True, stop=True)

    # ---- pack + AllReduce ----
    pack = ep_pool.tile([128, 66], f32, name="pack", tag="pack")
    nc.vector.memset(pack[:, :], 0.0)
    nc.vector.tensor_copy(pack[:, 0:64], acc_d[:, :])
    nc.vector.tensor_copy(pack[0:F, 64:65], psum_sv[:, :])
    nc.vector.tensor_copy(pack[0:F, 65:66], psum_qv[:, :])

    if stage == "loop":
        nc.sync.dma_start(y_dbg[:, :], pack[:, :])
        nc.sync.dma_start(y_out[:], pack[0, 0:64])
        for p in reversed(ctx_pools):
            p.__exit__(None, None, None)
        return

    cc_in = dram_pool.tile([128, 66], f32, name="cc_in", tag="cc_in")
    cc_out = dram_pool.tile([128, 66], f32, name="cc_out", tag="cc_out",
                            addr_space="Shared")
    nc.sync.dma_start(cc_in[:, :], pack[:, :])
    nc.gpsimd.collective_compute(
        "AllReduce",
        ADD,
        replica_groups=[list(range(NCORES))],
        ins=[cc_in[:, :].opt()],
        outs=[cc_out[:, :].opt()],
    )
    red = ep_pool.tile([128, 66], f32, name="red", tag="red")
    nc.sync.dma_start(red[:, :], cc_out[:, :])

    if stage == "pack":
        nc.sync.dma_start(y_dbg[:, :], red[:, :])
        nc.sync.dma_start(y_out[:], red[0, 0:64])
        for p in reversed(ctx_pools):
            p.__exit__(None, None, None)
        return

    # ---- replicated MLP head ----
    dseT = red[:, 0:64]          # [128 h1, 64 d] global masked sums
    ssum = red[0:F, 64:65]       # [64 f, 1] global state column sums
    ssq = red[0:F, 65:66]        # [64 f, 1] global state column sum-squares

    # state per-feature mean / 1/(std+eps), as [F,1] columns
    mean_s = ep_pool.tile([F, 1], f32, name="mean_s", tag="mean_s")
    nc.vector.tensor_scalar_mul(mean_s[:, :], ssum, 1.0 / N)
    ex2_s = ep_pool.tile([F, 1], f32, name="ex2_s", tag="ex2_s")
    nc.vector.tensor_scalar_mul(ex2_s[:, :], ssq, 1.0 / N)
    var_s = ep_pool.tile([F, 1], f32, name="var_s", tag="var_s")
    nc.vector.tensor_mul(var_s[:, :], mean_s[:, :], mean_s[:, :])
    nc.vector.tensor_sub(var_s[:, :], ex2_s[:, :], var_s[:, :])
    std_s = ep_pool.tile([F, 1], f32, name="std_s", tag="std_s")
    nc.scalar.activation(std_s[:, :], var_s[:, :], SQRT)
    nc.vector.tensor_scalar_add(std_s[:, :], std_s[:, :], EPS)
    inv_s = ep_pool.tile([F, 1], f32, name="inv_s", tag="inv_s")
    nc.vector.reciprocal(inv_s[:, :], std_s[:, :])

    # normalized state[pred], broadcast along free to [F, D], then
    # rep_latent.T = leaky(W2 @ xn + b2) computed for all D columns at once
    xn = ep_pool.tile([F, 1], f32, name="xn", tag="xn")
    nc.vector.tensor_scalar(xn[:, :], spred[:, :], mean_s[:, :], inv_s[:, :],
                            op0=SUB, op1=MUL)
    xn_b = ep_pool.tile([F, D], f32, name="xn_b", tag="xn_b")
    nc.scalar.activation(xn_b[:, :], zeros[0:F, :], IDENT, bias=xn[:, :])
    psum_repl = eppsum_pool.tile([H1, D], f32, name="psum_repl", tag="ep")
    nc.tensor.matmul(psum_repl[:, :], lhsT=w2T[:, :], rhs=xn_b[:, :],
                     start=True, stop=True)
    repl = ep_pool.tile([H1, D], f32, name="repl", tag="repl")
    nc.scalar.activation(repl[:, :], psum_repl[:, :], IDENT, bias=b2[:, :])
    repl_a = ep_pool.tile([H1, D], f32, name="repl_a", tag="repl_a")
    nc.vector.tensor_scalar_mul(repl_a[:, :], repl[:, :], SLOPE)
    nc.vector.tensor_max(repl[:, :], repl[:, :], repl_a[:, :])

    # device_feat_state normalization (over D, free axis) + embedding.
    # dfsT/w1T are zero-padded from 32 to 64 partitions host-side.
    mean_f = ep_pool.tile([64, 1], f32, name="mean_f", tag="mean_f")
    nc.vector.tensor_reduce(mean_f[:, :], dfsT[:, :], axis=AX, op=ADD)
    nc.vector.tensor_scalar_mul(mean_f[:, :], mean_f[:, :], 1.0 / D)
    sqf = ep_pool.tile([64, D], f32, name="sqf", tag="sqf")
    nc.scalar.activation(sqf[:, :], dfsT[:, :], SQUARE)
    qf = ep_pool.tile([64, 1], f32, name="qf", tag="qf")
    nc.vector.tensor_reduce(qf[:, :], sqf[:, :], axis=AX, op=ADD)
    nc.vector.tensor_scalar_mul(qf[:, :], qf[:, :], 1.0 / D)
    varf = ep_pool.tile([64, 1], f32, name="varf", tag="varf")
    nc.vector.tensor_mul(varf[:, :], mean_f[:, :], mean_f[:, :])
    nc.vector.tensor_sub(varf[:, :], qf[:, :], varf[:, :])
    stdf = ep_pool.tile([64, 1], f32, name="stdf", tag="stdf")
    nc.scalar.activation(stdf[:, :], varf[:, :], SQRT)
    nc.vector.tensor_scalar_add(stdf[:, :], stdf[:, :], EPS)
    invf = ep_pool.tile([64, 1], f32, name="invf", tag="invf")
    nc.vector.reciprocal(invf[:, :], stdf[:, :])
    dfsn = ep_pool.tile([64, D], f32, name="dfsn", tag="dfsn")
    nc.vector.tensor_scalar(dfsn[:, :], dfsT[:, :], mean_f[:, :], invf[:, :],
                            op0=SUB, op1=MUL)
    psum_dfe = eppsum_pool.tile([H1, D], f32, name="psum_dfe", tag="ep")
    nc.tensor.matmul(psum_dfe[:, :], lhsT=w1T[:, :], rhs=dfsn[:, :],
                     start=True, stop=True)
    dfeT = ep_pool.tile([H1, D], f32, name="dfeT", tag="dfeT")
    nc.scalar.activation(dfeT[:, :], psum_dfe[:, :], IDENT, bias=b1[:, :])
    dfe_a = ep_pool.tile([H1, D], f32, name="dfe_a", tag="dfe_a")
    nc.vector.tensor_scalar_mul(dfe_a[:, :], dfeT[:, :], SLOPE)
    nc.vector.tensor_max(dfeT[:, :], dfeT[:, :], dfe_a[:, :])

    # dse normalization (over D, free axis)
    mean_d = ep_pool.tile([H1, 1], f32, name="mean_d", tag="mean_d")
    nc.vector.tensor_reduce(mean_d[:, :], dseT, axis=AX, op=ADD)
    nc.vector.tensor_scalar_mul(mean_d[:, :], mean_d[:, :], 1.0 / D)
    sqd = ep_pool.tile([H1, D], f32, name="sqd", tag="sqd")
    nc.scalar.activation(sqd[:, :], dseT, SQUARE)
    qd = ep_pool.tile([H1, 1], f32, name="qd", tag="qd")
    nc.vector.tensor_reduce(qd[:, :], sqd[:, :], axis=AX, op=ADD)
    nc.vector.tensor_scalar_mul(qd[:, :], qd[:, :], 1.0 / D)
    vard = ep_pool.tile([H1, 1], f32, name="vard", tag="vard")
    nc.vector.tensor_mul(vard[:, :], mean_d[:, :], mean_d[:, :])
    nc.vector.tensor_sub(vard[:, :], qd[:, :], vard[:, :])
    stdd = ep_pool.tile([H1, 1], f32, name="stdd", tag="stdd")
    nc.scalar.activation(stdd[:, :], vard[:, :], SQRT)
    nc.vector.tensor_scalar_add(stdd[:, :], stdd[:, :], EPS)
    invd = ep_pool.tile([H1, 1], f32, name="invd", tag="invd")
    nc.vector.reciprocal(invd[:, :], stdd[:, :])
    dsen = ep_pool.tile([H1, D], f32, name="dsen", tag="dsen")
    nc.vector.tensor_scalar(dsen[:, :], dseT, mean_d[:, :], invd[:, :],
                            op0=SUB, op1=MUL)

    # broadcast mpnn[pred] along the D axis
    repe = ep_pool.tile([H1, D], f32, name="repe", tag="repe")
    nc.scalar.activation(repe[:, :], zeros[:, :], IDENT, bias=mpred[:, :])

    # h.T = leaky(W3 @ concat.T + b3): 4 accumulated chunks over c=512
    psum_h = eppsum_pool.tile([H2, D], f32, name="psum_h", tag="ep")
    chunks = [dfeT[:, :], repl[:, :], repe[:, :], dsen[:, :]]
    for k in range(4):
        nc.tensor.matmul(psum_h[:, :], lhsT=w3Tp[:, k * H2:(k + 1) * H2],
                         rhs=chunks[k], start=(k == 0), stop=(k == 3))
    hT = ep_pool.tile([H2, D], f32, name="hT", tag="hT")
    nc.scalar.activation(hT[:, :], psum_h[:, :], IDENT, bias=b3[:, :])
    hT_a = ep_pool.tile([H2, D], f32, name="hT_a", tag="hT_a")
    nc.vector.tensor_scalar_mul(hT_a[:, :], hT[:, :], SLOPE)
    nc.vector.tensor_max(hT[:, :], hT[:, :], hT_a[:, :])

    # output[d] = sum_j hT[j, d] * W4[0, j] + b4, as a [64, 1] column:
    # lhsT = hT [j, d] (K=64, M=64), rhs = w4T [64, 1] -> psum [64, 1]
    psum_o = eppsum_pool.tile([D, 1], f32, name="psum_o", tag="ep")
    nc.tensor.matmul(psum_o[:, :], lhsT=hT[:, :], rhs=w4T[:, :],
                     start=True, stop=True)
    out_sb = ep_pool.tile([D, 1], f32, name="out_sb", tag="out_sb")
    nc.scalar.activation(out_sb[:, :], psum_o[:, :], IDENT, bias=b4[:, :])
    nc.sync.dma_start(y_out[:], out_sb[:, 0])

    for p in reversed(ctx_pools):
        p.__exit__(None, None, None)


_compiled = None


def _get_compiled():
    global _compiled
    if _compiled is None:
        _compiled = build_program()
    return _compiled


def make_in_maps(inputs):
    state = np.ascontiguousarray(np.asarray(inputs["state"], dtype=np.float32))
    dfs = np.asarray(inputs["device_feat_state"], dtype=np.float32)
    mpnn = np.ascontiguousarray(np.asarray(inputs["mpnn_forward"], dtype=np.float32))
    W1 = np.asarray(inputs["W1"], dtype=np.float32)
    b1 = np.asarray(inputs["b1"], dtype=np.float32)
    W2 = np.asarray(inputs["W2"], dtype=np.float32)
    b2 = np.asarray(inputs["b2"], dtype=np.float32)
    W3 = np.asarray(inputs["W3"], dtype=np.float32)
    b3 = np.asarray(inputs["b3"], dtype=np.float32)
    W4 = np.asarray(inputs["W4"], dtype=np.float32)
    b4 = np.asarray(inputs["b4"], dtype=np.float32)
    mask = np.asarray(inputs["device_assign_state"])
    assert mask.dtype == np.int32
    pred = int(np.asarray(inputs["pred_node"]))

    w3Tp = np.ascontiguousarray(
        W3.T.reshape(4, H1, H2).transpose(1, 0, 2).reshape(H1, 4 * H2))
    common = {
        "x_dfsT": np.ascontiguousarray(np.pad(dfs.T, ((0, 64 - DF), (0, 0)))),
        "x_w1T": np.ascontiguousarray(np.pad(W1.T, ((0, 64 - DF), (0, 0)))),
        "x_b1": np.ascontiguousarray(b1.reshape(H1, 1)),
        "x_w2T": np.ascontiguousarray(W2.T),
        "x_b2": np.ascontiguousarray(b2.reshape(H1, 1)),
        "x_w3Tp": w3Tp,
        "x_b3": np.ascontiguousarray(b3.reshape(H2, 1)),
        "x_w4T": np.ascontiguousarray(W4.T),
        "x_b4": np.ascontiguousarray(np.broadcast_to(b4.reshape(1, 1), (D, 1))),
        "x_spred": np.ascontiguousarray(state[pred].reshape(F, 1)),
        "x_mpred": np.ascontiguousarray(mpnn[pred].reshape(H1, 1)),
    }
    in_maps = []
    for c in range(NCORES):
        sl = slice(c * NSH, (c + 1) * NSH)
        in_maps.append({
            **common,
            "x_state": state[sl],
            "x_mpnn": mpnn[sl],
            "x_mask": np.ascontiguousarray(mask[:, sl]),
        })
    return in_maps


def kernel(**inputs) -> np.ndarray:
    nc = _get_compiled()
    in_maps = make_in_maps(inputs)
    res = run_bass_kernel_spmd(nc, in_maps, core_ids=list(range(NCORES)))
    return np.asarray(res.results[0]["y_out"], dtype=np.float32)



# revision 2
# speedup vs baseline: 2.2007x; 2.2007x over previous
"""Trainium2 Bass kernel for nn_Device_Policy (segment_reduce).

Strategy (matches the sharding hint): shard the node axis N across 8
NeuronCores.  Each core holds a [N/8, 64] state shard, a [N/8, 128]
mpnn_forward shard and a [64, N/8] slice of the assignment mask.

All large inputs are staged host-side in bf16 and pre-laid-out so that
every SBUF tile loads with one big contiguous-per-partition DMA and the
mask arrives already node-major (partition = node % 128).  That removes
all on-chip transposes, casts and copies from the v1 kernel:
  - dse.T [128h, 64d] accumulates across all 256 K-blocks directly in
    one PSUM bank via bf16 matmuls (1 cycle/row vs 4 for fp32).
  - state column sums / sums-of-squares accumulate on DVE (bf16 add
    trees) + Act (square), then two tiny PE matmuls fold partitions.
The [128,64] dse.T partial plus the [64]+[64] state stats are packed
into one [128,66] f32 buffer and AllReduce'd across the 8 cores; every
core then runs the tiny replicated MLP head and writes the [64] output.
"""

import sys

if "/opt/trn_rl_repo" not in sys.path:
    sys.path.insert(0, "/opt/trn_rl_repo")

import ml_dtypes
import numpy as np

import concourse.bacc as bacc
import concourse.bass as bass
import concourse.mybir as mybir
import concourse.tile as tile
from concourse.bass_utils import run_bass_kernel_spmd

NCORES = 8
N = 262144
F = 64
D = 64
DF = 32
H1 = 128
H2 = 64
NSH = N // NCORES          # nodes per core = 32768
TILE = 4096                # nodes per loop tile
NT = NSH // TILE           # 8 tiles per core
BLK = TILE // 128          # 32 K-blocks (128 nodes each) per tile
EPS = 1e-6
SLOPE = 0.1

f32 = mybir.dt.float32
bf16 = mybir.dt.bfloat16
ADD = mybir.AluOpType.add
MUL = mybir.AluOpType.mult
SUB = mybir.AluOpType.subtract
AX = mybir.AxisListType.X
IDENT = mybir.ActivationFunctionType.Identity
SQUARE = mybir.ActivationFunctionType.Square
SQRT = mybir.ActivationFunctionType.Sqrt

NP_BF16 = ml_dtypes.bfloat16


def build_program():
    nc = bacc.Bacc(
        "TRN2",
        target_bir_lowering=False,
        debug=False,
        enable_asserts=False,
        num_devices=NCORES,
    )

    # big bf16 inputs, host-side pre-laid-out (see make_in_maps)
    x_mpnnL = nc.dram_tensor("x_mpnnL", [128, NT * BLK * 128], bf16,
                             kind="ExternalInput")
    x_maskL = nc.dram_tensor("x_maskL", [128, NT * BLK * 64], bf16,
                             kind="ExternalInput")
    x_stateL = nc.dram_tensor("x_stateL", [128, NT * BLK * 64], bf16,
                              kind="ExternalInput")
    # small f32 consts
    x_dfsT = nc.dram_tensor("x_dfsT", [64, D], f32, kind="ExternalInput")
    x_w1T = nc.dram_tensor("x_w1T", [64, H1], f32, kind="ExternalInput")
    x_b1 = nc.dram_tensor("x_b1", [H1, 1], f32, kind="ExternalInput")
    x_w2T = nc.dram_tensor("x_w2T", [F, H1], f32, kind="ExternalInput")
    x_b2 = nc.dram_tensor("x_b2", [H1, 1], f32, kind="ExternalInput")
    x_w3Tp = nc.dram_tensor("x_w3Tp", [H1, 4 * H2], f32, kind="ExternalInput")
    x_b3 = nc.dram_tensor("x_b3", [H2, 1], f32, kind="ExternalInput")
    x_w4T = nc.dram_tensor("x_w4T", [H2, 1], f32, kind="ExternalInput")
    x_b4 = nc.dram_tensor("x_b4", [D, 1], f32, kind="ExternalInput")
    x_spred = nc.dram_tensor("x_spred", [F, 1], f32, kind="ExternalInput")
    x_mpred = nc.dram_tensor("x_mpred", [H1, 1], f32, kind="ExternalInput")
    y_out = nc.dram_tensor("y_out", [D], f32, kind="ExternalOutput")

    with tile.TileContext(nc) as tc:
        emit(nc, tc, x_mpnnL, x_maskL, x_stateL, x_dfsT, x_w1T, x_b1, x_w2T,
             x_b2, x_w3Tp, x_b3, x_w4T, x_b4, x_spred, x_mpred, y_out)

    nc.compile()
    return nc


def emit(nc, tc, x_mpnnL, x_maskL, x_stateL, x_dfsT, x_w1T, x_b1, x_w2T, x_b2,
         x_w3Tp, x_b3, x_w4T, x_b4, x_spred, x_mpred, y_out):
    ctx_pools = []

    def pool(name, bufs, space="SBUF"):
        p = tc.tile_pool(name=name, bufs=bufs, space=space)
        ctx_pools.append(p)
        return p.__enter__()

    cpool = pool("const", 1)
    mp_pool = pool("mp", 3)
    mk_pool = pool("mk", 3)
    st_pool = pool("st", 2)
    sq_pool = pool("sq", 2)
    tmp_pool = pool("tmp", 2)
    acc_pool = pool("acc", 1)
    ep_pool = pool("ep", 1)
    dse_psum = pool("dsepsum", 1, space="PSUM")
    eppsum_pool = pool("eppsum", 2, space="PSUM")
    dram_pool = pool("dram", 1, space="DRAM")

    # ---- kick off the first big loop DMAs before anything else ----
    mp_tiles = []
    mk_tiles = []
    st_tiles = []

    def issue_tile_dmas(t):
        mp = mp_pool.tile([128, BLK * 128], bf16, name="mp", tag="mp")
        nc.sync.dma_start(mp[:, :], x_mpnnL[:, t * BLK * 128:(t + 1) * BLK * 128])
        mk = mk_pool.tile([128, BLK * 64], bf16, name="mk", tag="mk")
        nc.scalar.dma_start(mk[:, :], x_maskL[:, t * BLK * 64:(t + 1) * BLK * 64])
        st = st_pool.tile([128, BLK * 64], bf16, name="st", tag="st")
        nc.sync.dma_start(st[:, :], x_stateL[:, t * BLK * 64:(t + 1) * BLK * 64])
        mp_tiles.append(mp)
        mk_tiles.append(mk)
        st_tiles.append(st)

    issue_tile_dmas(0)

    # ---- consts on the gpsimd (SWDGE) queue: doesn't contend with the
    # big-load HWDGE queues ----
    dfsT = cpool.tile([64, D], f32, name="dfsT")
    nc.gpsimd.dma_start(dfsT[:, :], x_dfsT[:, :])
    w1T = cpool.tile([64, H1], f32, name="w1T")
    nc.gpsimd.dma_start(w1T[:, :], x_w1T[:, :])
    b1 = cpool.tile([H1, 1], f32, name="b1")
    nc.gpsimd.dma_start(b1[:, :], x_b1[:, :])
    w2T = cpool.tile([F, H1], f32, name="w2T")
    nc.gpsimd.dma_start(w2T[:, :], x_w2T[:, :])
    b2 = cpool.tile([H1, 1], f32, name="b2")
    nc.gpsimd.dma_start(b2[:, :], x_b2[:, :])
    w3Tp = cpool.tile([H1, 4 * H2], f32, name="w3Tp")
    nc.gpsimd.dma_start(w3Tp[:, :], x_w3Tp[:, :])
    b3 = cpool.tile([H2, 1], f32, name="b3")
    nc.gpsimd.dma_start(b3[:, :], x_b3[:, :])
    w4T = cpool.tile([H2, 1], f32, name="w4T")
    nc.gpsimd.dma_start(w4T[:, :], x_w4T[:, :])
    b4 = cpool.tile([D, 1], f32, name="b4")
    nc.gpsimd.dma_start(b4[:, :], x_b4[:, :])
    spred = cpool.tile([F, 1], f32, name="spred")
    nc.gpsimd.dma_start(spred[:, :], x_spred[:, :])
    mpred = cpool.tile([H1, 1], f32, name="mpred")
    nc.gpsimd.dma_start(mpred[:, :], x_mpred[:, :])

    issue_tile_dmas(1)

    # ---- small constants / accumulators ----
    ones_b = cpool.tile([128, 1], bf16, name="ones_b")
    nc.vector.memset(ones_b[:, :], 1.0)
    zeros = cpool.tile([128, D], f32, name="zeros")
    nc.vector.memset(zeros[:, :], 0.0)
    pack = cpool.tile([128, 66], f32, name="pack")
    nc.vector.memset(pack[:, :], 0.0)
    acc_s = acc_pool.tile([128, F], bf16, name="acc_s", tag="acc_s")
    acc_q = acc_pool.tile([128, F], bf16, name="acc_q", tag="acc_q")
    nc.vector.memset(acc_s[:, :], 0.0)
    nc.vector.memset(acc_q[:, :], 0.0)

    issue_tile_dmas(2)

    # ---- early head pieces that do not depend on the reduction:
    # device_feat embedding dfeT and the broadcast mpnn[pred] ----
    mean_f = ep_pool.tile([64, 1], f32, name="mean_f", tag="mean_f")
    nc.vector.tensor_reduce(mean_f[:, :], dfsT[:, :], axis=AX, op=ADD)
    nc.vector.tensor_scalar_mul(mean_f[:, :], mean_f[:, :], 1.0 / D)
    sqf = ep_pool.tile([64, D], f32, name="sqf", tag="sqf")
    nc.scalar.activation(sqf[:, :], dfsT[:, :], SQUARE)
    qf = ep_pool.tile([64, 1], f32, name="qf", tag="qf")
    nc.vector.tensor_reduce(qf[:, :], sqf[:, :], axis=AX, op=ADD)
    nc.vector.tensor_scalar_mul(qf[:, :], qf[:, :], 1.0 / D)
    varf = ep_pool.tile([64, 1], f32, name="varf", tag="varf")
    nc.vector.tensor_mul(varf[:, :], mean_f[:, :], mean_f[:, :])
    nc.vector.tensor_sub(varf[:, :], qf[:, :], varf[:, :])
    stdf = ep_pool.tile([64, 1], f32, name="stdf", tag="stdf")
    nc.scalar.activation(stdf[:, :], varf[:, :], SQRT)
    nc.vector.tensor_scalar_add(stdf[:, :], stdf[:, :], EPS)
    invf = ep_pool.tile([64, 1], f32, name="invf", tag="invf")
    nc.vector.reciprocal(invf[:, :], stdf[:, :])
    dfsn = ep_pool.tile([64, D], f32, name="dfsn", tag="dfsn")
    nc.vector.tensor_scalar(dfsn[:, :], dfsT[:, :], mean_f[:, :], invf[:, :],
                            op0=SUB, op1=MUL)
    psum_dfe = eppsum_pool.tile([H1, D], f32, name="psum_dfe", tag="ep")
    nc.tensor.matmul(psum_dfe[:, :], lhsT=w1T[:, :], rhs=dfsn[:, :],
                     start=True, stop=True)
    dfeT = ep_pool.tile([H1, D], f32, name="dfeT", tag="dfeT")
    nc.scalar.activation(dfeT[:, :], psum_dfe[:, :], IDENT, bias=b1[:, :])
    dfe_a = ep_pool.tile([H1, D], f32, name="dfe_a", tag="dfe_a")
    nc.vector.tensor_scalar_mul(dfe_a[:, :], dfeT[:, :], SLOPE)
    nc.vector.tensor_max(dfeT[:, :], dfeT[:, :], dfe_a[:, :])

    repe = ep_pool.tile([H1, D], f32, name="repe", tag="repe")
    nc.scalar.activation(repe[:, :], zeros[:, :], IDENT, bias=mpred[:, :])

    # ---- main loop over node tiles of TILE=4096 ----
    psum_dse = dse_psum.tile([H1, D], f32, name="psum_dse", tag="psum_dse")

    def addtree(src, acc, pfx):
        # [128, 2048] -> [128, 64] dense bf16 add tree, then acc +=
        t1 = tmp_pool.tile([128, 1024], bf16, name=f"{pfx}1", tag=f"{pfx}1")
        nc.vector.tensor_add(t1[:, :], src[:, 0:1024], src[:, 1024:2048])
        t2 = tmp_pool.tile([128, 512], bf16, name=f"{pfx}2", tag=f"{pfx}2")
        nc.vector.tensor_add(t2[:, :], t1[:, 0:512], t1[:, 512:1024])
        t3 = tmp_pool.tile([128, 256], bf16, name=f"{pfx}3", tag=f"{pfx}3")
        nc.vector.tensor_add(t3[:, :], t2[:, 0:256], t2[:, 256:512])
        t4 = tmp_pool.tile([128, 128], bf16, name=f"{pfx}4", tag=f"{pfx}4")
        nc.vector.tensor_add(t4[:, :], t3[:, 0:128], t3[:, 128:256])
        t5 = tmp_pool.tile([128, 64], bf16, name=f"{pfx}5", tag=f"{pfx}5")
        nc.vector.tensor_add(t5[:, :], t4[:, 0:64], t4[:, 64:128])
        nc.vector.tensor_add(acc[:, :], acc[:, :], t5[:, :])

    for t in range(NT):
        if t + 3 < NT:
            issue_tile_dmas(t + 3)
        mp = mp_tiles[t]
        mk = mk_tiles[t]
        st = st_tiles[t]

        for b in range(BLK):
            nc.tensor.matmul(
                psum_dse[:, :],
                lhsT=mp[:, b * 128:(b + 1) * 128],
                rhs=mk[:, b * 64:(b + 1) * 64],
                start=(t == 0 and b == 0),
                stop=(t == NT - 1 and b == BLK - 1),
            )

        sq = sq_pool.tile([128, BLK * 64], bf16, name="sq", tag="sq")
        nc.scalar.activation(sq[:, :], st[:, :], SQUARE)
        addtree(st, acc_s, "ts")
        addtree(sq, acc_q, "tq")

    # ---- fold partitions of the state stats via PE ----
    psum_sv = eppsum_pool.tile([F, 1], f32, name="psum_sv", tag="ep")
    nc.tensor.matmul(psum_sv[:, :], lhsT=acc_s[:, :], rhs=ones_b[:, :],
                     start=True, stop=True)
    psum_qv = eppsum_pool.tile([F, 1], f32, name="psum_qv", tag="ep")
    nc.tensor.matmul(psum_qv[:, :], lhsT=acc_q[:, :], rhs=ones_b[:, :],
                     start=True, stop=True)

    # ---- pack + AllReduce ----
    nc.vector.tensor_copy(pack[:, 0:64], psum_dse[:, :])
    nc.vector.tensor_copy(pack[0:F, 64:65], psum_sv[:, :])
    nc.vector.tensor_copy(pack[0:F, 65:66], psum_qv[:, :])

    cc_in = dram_pool.tile([128, 66], f32, name="cc_in", tag="cc_in")
    cc_out = dram_pool.tile([128, 66], f32, name="cc_out", tag="cc_out",
                            addr_space="Shared")
    nc.sync.dma_start(cc_in[:, :], pack[:, :])
    nc.gpsimd.collective_compute(
        "AllReduce",
        ADD,
        replica_groups=[list(range(NCORES))],
        ins=[cc_in[:, :].opt()],
        outs=[cc_out[:, :].opt()],
    )
    red = ep_pool.tile([128, 66], f32, name="red", tag="red")
    nc.sync.dma_start(red[:, :], cc_out[:, :])

    # ---- replicated MLP head ----
    dseT = red[:, 0:64]          # [128 h1, 64 d] global masked sums
    ssum = red[0:F, 64:65]       # [64 f, 1] global state column sums
    ssq = red[0:F, 65:66]        # [64 f, 1] global state column sum-squares

    # state per-feature mean / 1/(std+eps), as [F,1] columns
    mean_s = ep_pool.tile([F, 1], f32, name="mean_s", tag="mean_s")
    nc.vector.tensor_scalar_mul(mean_s[:, :], ssum, 1.0 / N)
    ex2_s = ep_pool.tile([F, 1], f32, name="ex2_s", tag="ex2_s")
    nc.vector.tensor_scalar_mul(ex2_s[:, :], ssq, 1.0 / N)
    var_s = ep_pool.tile([F, 1], f32, name="var_s", tag="var_s")
    nc.vector.tensor_mul(var_s[:, :], mean_s[:, :], mean_s[:, :])
    nc.vector.tensor_sub(var_s[:, :], ex2_s[:, :], var_s[:, :])
    std_s = ep_pool.tile([F, 1], f32, name="std_s", tag="std_s")
    nc.scalar.activation(std_s[:, :], var_s[:, :], SQRT)
    nc.vector.tensor_scalar_add(std_s[:, :], std_s[:, :], EPS)
    inv_s = ep_pool.tile([F, 1], f32, name="inv_s", tag="inv_s")
    nc.vector.reciprocal(inv_s[:, :], std_s[:, :])

    # normalized state[pred], broadcast along free to [F, D], then
    # rep_latent.T = leaky(W2 @ xn + b2) computed for all D columns at once
    xn = ep_pool.tile([F, 1], f32, name="xn", tag="xn")
    nc.vector.tensor_scalar(xn[:, :], spred[:, :], mean_s[:, :], inv_s[:, :],
                            op0=SUB, op1=MUL)
    xn_b = ep_pool.tile([F, D], f32, name="xn_b", tag="xn_b")
    nc.scalar.activation(xn_b[:, :], zeros[0:F, :], IDENT, bias=xn[:, :])
    psum_repl = eppsum_pool.tile([H1, D], f32, name="psum_repl", tag="ep")
    nc.tensor.matmul(psum_repl[:, :], lhsT=w2T[:, :], rhs=xn_b[:, :],
                     start=True, stop=True)
    repl = ep_pool.tile([H1, D], f32, name="repl", tag="repl")
    nc.scalar.activation(repl[:, :], psum_repl[:, :], IDENT, bias=b2[:, :])
    repl_a = ep_pool.tile([H1, D], f32, name="repl_a", tag="repl_a")
    nc.vector.tensor_scalar_mul(repl_a[:, :], repl[:, :], SLOPE)
    nc.vector.tensor_max(repl[:, :], repl[:, :], repl_a[:, :])

    # dse normalization (over D, free axis)
    mean_d = ep_pool.tile([H1, 1], f32, name="mean_d", tag="mean_d")
    nc.vector.tensor_reduce(mean_d[:, :], dseT, axis=AX, op=ADD)
    nc.vector.tensor_scalar_mul(mean_d[:, :], mean_d[:, :], 1.0 / D)
    sqd = ep_pool.tile([H1, D], f32, name="sqd", tag="sqd")
    nc.scalar.activation(sqd[:, :], dseT, SQUARE)
    qd = ep_pool.tile([H1, 1], f32, name="qd", tag="qd")
    nc.vector.tensor_reduce(qd[:, :], sqd[:, :], axis=AX, op=ADD)
    nc.vector.tensor_scalar_mul(qd[:, :], qd[:, :], 1.0 / D)
    vard = ep_pool.tile([H1, 1], f32, name="vard", tag="vard")
    nc.vector.tensor_mul(vard[:, :], mean_d[:, :], mean_d[:, :])
    nc.vector.tensor_sub(vard[:, :], qd[:, :], vard[:, :])
    stdd = ep_pool.tile([H1, 1], f32, name="stdd", tag="stdd")
    nc.scalar.activation(stdd[:, :], vard[:, :], SQRT)
    nc.vector.tensor_scalar_add(stdd[:, :], stdd[:, :], EPS)
    invd = ep_pool.tile([H1, 1], f32, name="invd", tag="invd")
    nc.vector.reciprocal(invd[:, :], stdd[:, :])
    dsen = ep_pool.tile([H1, D], f32, name="dsen", tag="dsen")
    nc.vector.tensor_scalar(dsen[:, :], dseT, mean_d[:, :], invd[:, :],
                            op0=SUB, op1=MUL)

    # h.T = leaky(W3 @ concat.T + b3): 4 accumulated chunks over c=512
    psum_h = eppsum_pool.tile([H2, D], f32, name="psum_h", tag="ep")
    chunks = [dfeT[:, :], repl[:, :], repe[:, :], dsen[:, :]]
    for k in range(4):
        nc.tensor.matmul(psum_h[:, :], lhsT=w3Tp[:, k * H2:(k + 1) * H2],
                         rhs=chunks[k], start=(k == 0), stop=(k == 3))
    hT = ep_pool.tile([H2, D], f32, name="hT", tag="hT")
    nc.scalar.activation(hT[:, :], psum_h[:, :], IDENT, bias=b3[:, :])
    hT_a = ep_pool.tile([H2, D], f32, name="hT_a", tag="hT_a")
    nc.vector.tensor_scalar_mul(hT_a[:, :], hT[:, :], SLOPE)
    nc.vector.tensor_max(hT[:, :], hT[:, :], hT_a[:, :])

    # output[d] = sum_j hT[j, d] * W4[0, j] + b4, as a [64, 1] column
    psum_o = eppsum_pool.tile([D, 1], f32, name="psum_o", tag="ep")
    nc.tensor.matmul(psum_o[:, :], lhsT=hT[:, :], rhs=w4T[:, :],
                     start=True, stop=True)
    out_sb = ep_pool.tile([D, 1], f32, name="out_sb", tag="out_sb")
    nc.scalar.activation(out_sb[:, :], psum_o[:, :], IDENT, bias=b4[:, :])
    nc.sync.dma_start(y_out[:], out_sb[:, 0])

    for p in reversed(ctx_pools):
        p.__exit__(None, None, None)


_compiled = None


def _get_compiled():
    global _compiled
    if _compiled is None:
        _compiled = build_program()
    return _compiled


def make_in_maps(inputs):
    state = np.asarray(inputs["state"], dtype=np.float32)
    dfs = np.asarray(inputs["device_feat_state"], dtype=np.float32)
    mpnn = np.asarray(inputs["mpnn_forward"], dtype=np.float32)
    W1 = np.asarray(inputs["W1"], dtype=np.float32)
    b1 = np.asarray(inputs["b1"], dtype=np.float32)
    W2 = np.asarray(inputs["W2"], dtype=np.float32)
    b2 = np.asarray(inputs["b2"], dtype=np.float32)
    W3 = np.asarray(inputs["W3"], dtype=np.float32)
    b3 = np.asarray(inputs["b3"], dtype=np.float32)
    W4 = np.asarray(inputs["W4"], dtype=np.float32)
    b4 = np.asarray(inputs["b4"], dtype=np.float32)
    mask = np.asarray(inputs["device_assign_state"])
    assert mask.dtype == np.int32
    pred = int(np.asarray(inputs["pred_node"]))

    w3Tp = np.ascontiguousarray(
        W3.T.reshape(4, H1, H2).transpose(1, 0, 2).reshape(H1, 4 * H2))
    common = {
        "x_dfsT": np.ascontiguousarray(np.pad(dfs.T, ((0, 64 - DF), (0, 0)))),
        "x_w1T": np.ascontiguousarray(np.pad(W1.T, ((0, 64 - DF), (0, 0)))),
        "x_b1": np.ascontiguousarray(b1.reshape(H1, 1)),
        "x_w2T": np.ascontiguousarray(W2.T),
        "x_b2": np.ascontiguousarray(b2.reshape(H1, 1)),
        "x_w3Tp": w3Tp,
        "x_b3": np.ascontiguousarray(b3.reshape(H2, 1)),
        "x_w4T": np.ascontiguousarray(W4.T),
        "x_b4": np.ascontiguousarray(np.broadcast_to(b4.reshape(1, 1), (D, 1))),
        "x_spred": np.ascontiguousarray(state[pred].reshape(F, 1)),
        "x_mpred": np.ascontiguousarray(mpnn[pred].reshape(H1, 1)),
    }

    # bf16 casts of the big tensors (mask values 0/1 are exact in bf16)
    mpnn16 = mpnn.astype(NP_BF16)
    state16 = state.astype(NP_BF16)
    mask16 = mask.astype(NP_BF16)

    in_maps = []
    for c in range(NCORES):
        sl = slice(c * NSH, (c + 1) * NSH)
        # node n (local) = t*TILE + b*128 + p lives at [p, (t*BLK + b)*w + j]
        mpnnL = np.ascontiguousarray(
            mpnn16[sl].reshape(NT, BLK, 128, 128)
            .transpose(2, 0, 1, 3).reshape(128, NT * BLK * 128))
        stateL = np.ascontiguousarray(
            state16[sl].reshape(NT, BLK, 128, F)
            .transpose(2, 0, 1, 3).reshape(128, NT * BLK * F))
        maskL = np.ascontiguousarray(
            mask16[:, sl].reshape(D, NT, BLK, 128)
            .transpose(3, 1, 2, 0).reshape(128, NT * BLK * D))
        in_maps.append({
            **common,
            "x_mpnnL": mpnnL,
            "x_maskL": maskL,
            "x_stateL": stateL,
        })
    return in_maps


def kernel(**inputs) -> np.ndarray:
    nc = _get_compiled()
    in_maps = make_in_maps(inputs)
    res = run_bass_kernel_spmd(nc, in_maps, core_ids=list(range(NCORES)))
    return np.asarray(res.results[0]["y_out"], dtype=np.float32)
